# revision 6
# baseline (speedup 1.0000x reference)
"""Naive multi-head attention on 8 Trainium2 NeuronCores.

Sharding: one attention head per core (H=8, tensor-parallel on heads).
Each core computes, for its head h:
    qT = wq.T @ xt            [64, S]   (d on partitions)
    kT = wk.T @ xt            [64, S]
    v  = xt.T @ wv            [S, 64]   (s on partitions)
    scoresT[k, q] = (kT tile).T @ qT    (keys on partitions, queries free)
    attnT = exp(scoresT * SCALE)        (no max subtraction: |scores*SCALE| < ~10)
    outT_ext = [v | 1].T-contracted: psum[65, qc] accumulating over key tiles
        rows 0..63 = unnormalized out.T, row 64 = softmax denominators
    y_partial[s, :] = (outT[:, s].T @ woT) / denom[s]
Host sums the 8 per-core partial projections (Megatron row-parallel reduce).

Compute dtype: fp32 data with float32r matmuls (full-rate on TRN2 for
moving-dim >= 256), exp in fp32 on the scalar engine.
"""

import sys

for _p in ("/opt/trn_rl_repo",):
    if _p not in sys.path:
        sys.path.insert(0, _p)

from contextlib import ExitStack

import numpy as np

import concourse.bass as bass  # noqa: F401  (engine types referenced via nc)
import concourse.tile as tile
from concourse import bacc, mybir
from concourse.bass_utils import run_bass_kernel_spmd

F32 = mybir.dt.float32
F32R = mybir.dt.float32r
EXP = mybir.ActivationFunctionType.Exp

S = 4096  # sequence length
D = 512  # model dim
H = 8  # heads
HD = 64  # head dim
SCALE = 0.125  # 1 / sqrt(HD)
N_CORES = 8


def _r(ap):
    return ap.bitcast(F32R)


def attention_kernel(tc, outs, ins, s=S):
    """Build the per-core program. ins/outs are dicts of DRAM APs."""
    nc = tc.nc
    xt_d, wq_d, wk_d, wv_d, wo_d = (
        ins["xt"],
        ins["wq"],
        ins["wk"],
        ins["wv"],
        ins["wo"],
    )
    y_d = outs["y"]

    qcw = min(1024, s)  # query-chunk width (scoresT free dim)
    nqc = s // qcw  # query chunks
    nkt = s // 128  # key tiles
    nmt = D // 128  # model-dim tiles (contraction for projections)
    n_qmm = qcw // 512 if qcw >= 512 else 1  # N=512 matmuls per chunk width
    qmm_w = min(512, qcw)

    with ExitStack() as ctx:
        persist = ctx.enter_context(tc.tile_pool(name="persist", bufs=1))
        scores_ps = ctx.enter_context(
            tc.tile_pool(name="scores_ps", bufs=2, space="PSUM")
        )
        out_ps = ctx.enter_context(tc.tile_pool(name="out_ps", bufs=1, space="PSUM"))
        y_ps = ctx.enter_context(tc.tile_pool(name="y_ps", bufs=1, space="PSUM"))
        attn_pool = ctx.enter_context(tc.tile_pool(name="attn", bufs=3))
        outw_pool = ctx.enter_context(tc.tile_pool(name="outw", bufs=2))
        y_sb_pool = ctx.enter_context(tc.tile_pool(name="y_sb", bufs=3))
        small_pool = ctx.enter_context(tc.tile_pool(name="small", bufs=2))
        dram_pool = ctx.enter_context(tc.tile_pool(name="dram", bufs=2, space="DRAM"))

        # ---- load inputs --------------------------------------------------
        xt_sb = []
        for mt in range(nmt):
            t = persist.tile([128, s], F32R, tag=f"xt{mt}")
            nc.sync.dma_start(out=t, in_=_r(xt_d[mt * 128 : (mt + 1) * 128, :]))
            xt_sb.append(t)
        wq_sb = persist.tile([128, nmt, HD], F32R, tag="wq")
        nc.sync.dma_start(out=wq_sb, in_=_r(wq_d).rearrange("(t p) d -> p t d", p=128))
        wk_sb = persist.tile([128, nmt, HD], F32R, tag="wk")
        nc.sync.dma_start(out=wk_sb, in_=_r(wk_d).rearrange("(t p) d -> p t d", p=128))
        wv_sb = persist.tile([128, nmt, HD], F32R, tag="wv")
        nc.sync.dma_start(out=wv_sb, in_=_r(wv_d).rearrange("(t p) d -> p t d", p=128))
        wo_sb = persist.tile([HD, D], F32R, tag="wo")
        nc.sync.dma_start(out=wo_sb, in_=_r(wo_d))

        one_d = ins["one"]

        # ---- q/k projections: qT/kT [64, s] (head dim on partitions) -----
        qt_sb = persist.tile([HD, s], F32R, tag="qt")
        kt_sb = persist.tile([HD, s], F32R, tag="kt")
        nch = s // 512
        for w_sb, t_sb in ((wq_sb, qt_sb), (wk_sb, kt_sb)):
            for ch in range(nch):
                ps = scores_ps.tile([HD, 512], F32, tag="ps")
                for mt in range(nmt):
                    nc.tensor.matmul(
                        ps,
                        w_sb[:, mt, :],
                        xt_sb[mt][:, ch * 512 : (ch + 1) * 512],
                        start=(mt == 0),
                        stop=(mt == nmt - 1),
                    )
                nc.vector.tensor_copy(t_sb[:, ch * 512 : (ch + 1) * 512], ps)

        # ---- v projection, direct [s, 64] layout, plus ones column -------
        # von[:, kt, 0:64] = v rows, von[:, kt, 64] = 1.0 (denominator trick)
        von = persist.tile([128, nkt, HD + 1], F32R, tag="von")
        one_bcast = bass.AP(
            tensor=_r(one_d).tensor,
            offset=_r(one_d).offset,
            ap=[[0, 128], [0, nkt], [0, 1]],
        )
        nc.gpsimd.dma_start(out=von[:, :, HD : HD + 1], in_=one_bcast)
        for st in range(nkt):
            ps = scores_ps.tile([128, HD], F32, tag="ps")
            for mt in range(nmt):
                nc.tensor.matmul(
                    ps,
                    xt_sb[mt][:, st * 128 : (st + 1) * 128],
                    wv_sb[:, mt, :],
                    start=(mt == 0),
                    stop=(mt == nmt - 1),
                )
            nc.vector.tensor_copy(von[:, st, 0:HD], ps)

        # ---- main loop over query chunks ---------------------------------
        for qc in range(nqc):
            q0 = qc * qcw
            psum_o = out_ps.tile([HD + 1, qcw], F32, tag="po")
            for kt in range(nkt):
                psum_s = scores_ps.tile([128, qcw], F32, tag="ps")
                for j in range(n_qmm):
                    nc.tensor.matmul(
                        psum_s[:, j * qmm_w : (j + 1) * qmm_w],
                        kt_sb[:, kt * 128 : (kt + 1) * 128],
                        qt_sb[:, q0 + j * qmm_w : q0 + (j + 1) * qmm_w],
                        start=True,
                        stop=True,
                    )
                attn_t = attn_pool.tile([128, qcw], F32R, tag="attn")
                nc.scalar.activation(out=attn_t, in_=psum_s, func=EXP, scale=SCALE)
                for j in range(n_qmm):
                    nc.tensor.matmul(
                        psum_o[:, j * qmm_w : (j + 1) * qmm_w],
                        von[:, kt, :],
                        attn_t[:, j * qmm_w : (j + 1) * qmm_w],
                        start=(kt == 0),
                        stop=(kt == nkt - 1),
                    )

            # evacuate outT (+denominator row) and normalize via out proj
            outw = outw_pool.tile([HD + 1, qcw], F32R, tag="outw")
            nc.vector.tensor_copy(outw, psum_o)
            nst_c = qcw // 128
            # transpose the denominator row [1, qcw] -> [128, nst_c]:
            # bounce through DRAM (partition<->free remap is free there)
            den_dram = dram_pool.tile([1, qcw], F32, tag="den_dram")
            nc.sync.dma_start(out=den_dram, in_=outw[HD : HD + 1, :].bitcast(F32))
            den_col = small_pool.tile([128, nst_c], F32, tag="den_col")
            nc.sync.dma_start(
                out=den_col,
                in_=den_dram.rearrange("p (c q) -> (p q) c", q=128),
            )
            recip = small_pool.tile([128, nst_c], F32, tag="recip")
            nc.vector.reciprocal(recip, den_col)
            for t in range(nst_c):
                yp = y_ps.tile([128, D], F32, tag="yp")
                nc.tensor.matmul(
                    yp,
                    outw[0:HD, t * 128 : (t + 1) * 128],
                    wo_sb,
                    start=True,
                    stop=True,
                )
                y_sb = y_sb_pool.tile([128, D], F32, tag="y_sb")
                nc.vector.tensor_scalar_mul(y_sb, yp, recip[:, t : t + 1])
                row0 = q0 + t * 128
                nc.sync.dma_start(out=y_d[row0 : row0 + 128, :], in_=y_sb)


def build_program(s=S):
    nc = bacc.Bacc(
        "TRN2", target_bir_lowering=False, debug=False, num_devices=N_CORES
    )
    ins = {
        "xt": nc.dram_tensor("xt", [D, s], F32R, kind="ExternalInput").ap(),
        "wq": nc.dram_tensor("wq", [D, HD], F32R, kind="ExternalInput").ap(),
        "wk": nc.dram_tensor("wk", [D, HD], F32R, kind="ExternalInput").ap(),
        "wv": nc.dram_tensor("wv", [D, HD], F32R, kind="ExternalInput").ap(),
        "wo": nc.dram_tensor("wo", [HD, D], F32R, kind="ExternalInput").ap(),
        "one": nc.dram_tensor("one", [1, 1], F32R, kind="ExternalInput").ap(),
    }
    outs = {"y": nc.dram_tensor("y", [s, D], F32, kind="ExternalOutput").ap()}
    with tile.TileContext(nc) as tc:
        attention_kernel(tc, outs, ins, s=s)
    nc.compile()
    return nc


_NC_CACHE = {}


def make_in_maps(x, Wq, Wk, Wv, Wo):
    """Host-side sharding: slice per-head weights, transpose layouts."""
    xt = np.ascontiguousarray(x.reshape(-1, D).T).astype(np.float32)
    in_maps = []
    for h in range(N_CORES):
        r = slice(h * HD, (h + 1) * HD)
        in_maps.append(
            {
                "xt": xt,
                "wq": np.ascontiguousarray(Wq[r, :].T).astype(np.float32),
                "wk": np.ascontiguousarray(Wk[r, :].T).astype(np.float32),
                "wv": np.ascontiguousarray(Wv[r, :].T).astype(np.float32),
                "wo": np.ascontiguousarray(Wo[:, r].T).astype(np.float32),
                "one": np.ones((1, 1), dtype=np.float32),
            }
        )
    return in_maps


def run(x, Wq, Wk, Wv, Wo, trace=False):
    if S not in _NC_CACHE:
        _NC_CACHE[S] = build_program(S)
    nc = _NC_CACHE[S]
    in_maps = make_in_maps(x, Wq, Wk, Wv, Wo)
    res = run_bass_kernel_spmd(
        nc, in_maps, core_ids=list(range(N_CORES)), trace=trace
    )
    y = res.results[0]["y"].astype(np.float64)
    for c in range(1, N_CORES):
        y += res.results[c]["y"]
    return y.astype(np.float32).reshape(1, S, D), res


def kernel(x, Wq, Wk, Wv, Wo):
    y, _ = run(
        np.asarray(x, dtype=np.float32),
        np.asarray(Wq, dtype=np.float32),
        np.asarray(Wk, dtype=np.float32),
        np.asarray(Wv, dtype=np.float32),
        np.asarray(Wo, dtype=np.float32),
    )
    return y


# revision 14
# speedup vs baseline: 1.5808x; 1.5808x over previous
"""Naive multi-head attention on 8 Trainium2 NeuronCores.

Sharding: one attention head per core (H=8, tensor-parallel on heads).
Each core computes, for its head h:
    qT = wq.T @ xt            [64, S]   (d on partitions)
    kT = wk.T @ xt            [64, S]
    v  = xt.T @ wv            [S, 64]   (s on partitions)
    scoresT[k, q] = (kT tile).T @ qT    (keys on partitions, queries free)
    attnT = exp(scoresT * SCALE)        (no max subtraction: |scores*SCALE| < ~10)
    outT_ext = [v | 1].T-contracted: psum[65, qc] accumulating over key tiles
        rows 0..63 = unnormalized out.T, row 64 = softmax denominators
    y_partial[s, :] = (outT[:, s].T @ woT) / denom[s]
Host sums the 8 per-core partial projections (Megatron row-parallel reduce).

Compute dtype: fp32 data with float32r matmuls (full-rate on TRN2 for
moving-dim >= 256), exp in fp32 on the scalar engine.
"""

import sys

for _p in ("/opt/trn_rl_repo",):
    if _p not in sys.path:
        sys.path.insert(0, _p)

from contextlib import ExitStack

import numpy as np

import concourse.bass as bass  # noqa: F401  (engine types referenced via nc)
import concourse.tile as tile
from concourse import bacc, mybir
from concourse.bass_utils import run_bass_kernel_spmd

F32 = mybir.dt.float32
F32R = mybir.dt.float32r
EXP = mybir.ActivationFunctionType.Exp

S = 4096  # sequence length
D = 512  # model dim
H = 8  # heads
HD = 64  # head dim
SCALE = 0.125  # 1 / sqrt(HD)
N_CORES = 8


def _r(ap):
    return ap.bitcast(F32R)


def attention_kernel(tc, outs, ins, s=S):
    """Build the per-core program. ins/outs are dicts of DRAM APs.

    All matmuls use K=128 contractions (zero-padded where the math needs
    only 64) — K<128 matmuls do not register as PE activity for the HAM
    clock gate, which otherwise locks the PE at 1.2 GHz for the whole
    main loop (measured: 427ns vs 229ns per N=512 f32r matmul).

    The kt loop is software-pipelined: PV for key tile kt-1 issues after
    QK for kt so the in-order PE queue never waits on exp(kt); the
    output projection of each query chunk is deferred into the first
    iterations of the next chunk's loop.
    """
    nc = tc.nc
    xt_d, wq_d, wk_d, wv_d, wo_d = (
        ins["xt"],
        ins["wq"],
        ins["wk"],
        ins["wv"],
        ins["wo"],
    )
    z64_d = ins["zeros64"]
    vinit_d = ins["vinit"]
    y_d = outs["y"]

    qcw = min(1024, s)  # query-chunk width (scoresT free dim)
    nqc = s // qcw  # query chunks
    nkt = s // 128  # key tiles
    nmt = D // 128  # model-dim tiles (contraction for projections)
    n_qmm = qcw // 512 if qcw >= 512 else 1  # N=512 matmuls per chunk width
    qmm_w = min(512, qcw)
    nch = s // 512

    with ExitStack() as ctx:
        persist = ctx.enter_context(tc.tile_pool(name="persist", bufs=1))
        scores_ps = ctx.enter_context(
            tc.tile_pool(name="scores_ps", bufs=2, space="PSUM")
        )
        out_ps = ctx.enter_context(tc.tile_pool(name="out_ps", bufs=1, space="PSUM"))
        y_ps = ctx.enter_context(tc.tile_pool(name="y_ps", bufs=2, space="PSUM"))
        attn_pool = ctx.enter_context(tc.tile_pool(name="attn", bufs=4))
        outw_pool = ctx.enter_context(tc.tile_pool(name="outw", bufs=2))
        y_sb_pool = ctx.enter_context(tc.tile_pool(name="y_sb", bufs=4))
        small_pool = ctx.enter_context(tc.tile_pool(name="small", bufs=2))
        dram_pool = ctx.enter_context(tc.tile_pool(name="dram", bufs=2, space="DRAM"))

        # ---- warm the exp activation table immediately -------------------
        warm_t = small_pool.tile([1, 16], F32, tag="warm")
        nc.vector.memset(warm_t, 0.0)
        warm_o = small_pool.tile([1, 16], F32, tag="warmo")
        nc.scalar.activation(out=warm_o, in_=warm_t, func=EXP)

        # ---- static tiles: weights, zero pads (fast contiguous DMAs) -----
        wq_sb = persist.tile([128, nmt, HD], F32R, tag="wq")
        nc.sync.dma_start(out=wq_sb, in_=_r(wq_d).rearrange("(t p) d -> p t d", p=128))
        wk_sb = persist.tile([128, nmt, HD], F32R, tag="wk")
        nc.sync.dma_start(out=wk_sb, in_=_r(wk_d).rearrange("(t p) d -> p t d", p=128))
        wv_sb = persist.tile([128, nmt, HD], F32R, tag="wv")
        nc.sync.dma_start(out=wv_sb, in_=_r(wv_d).rearrange("(t p) d -> p t d", p=128))
        # wo padded to K=128: rows 0..63 = woT, rows 64..127 = 0
        wo_sb = persist.tile([128, D], F32R, tag="wo")
        nc.sync.dma_start(out=wo_sb[0:HD, :], in_=_r(wo_d))
        nc.sync.dma_start(out=wo_sb[HD:128, :], in_=_r(z64_d[:, 0:D]))

        # qt0 = [qT; 0], kt0 = [kT; 0]; von[:, kt] = [v | 1 | 0...]
        qt0 = persist.tile([128, s], F32R, tag="qt0")
        kt0 = persist.tile([128, s], F32R, tag="kt0")
        nc.sync.dma_start(out=qt0[HD:128, :], in_=_r(z64_d))
        nc.sync.dma_start(out=kt0[HD:128, :], in_=_r(z64_d))
        von = persist.tile([128, nkt, 128], F32R, tag="von")
        nc.sync.dma_start(out=von[:, :, HD:128], in_=_r(vinit_d))

        # ---- xt in column chunks; projections interleave per chunk -------
        xt_sb = [
            persist.tile([128, s], F32R, tag=f"xt{mt}", name=f"xt_sb{mt}")
            for mt in range(nmt)
        ]
        for ch in range(nch):
            cs = slice(ch * 512, (ch + 1) * 512)
            for mt in range(nmt):
                nc.sync.dma_start(
                    out=xt_sb[mt][:, cs], in_=_r(xt_d[mt * 128 : (mt + 1) * 128, cs])
                )
            for w_sb, t_sb in ((wq_sb, qt0), (wk_sb, kt0)):
                ps = scores_ps.tile([HD, 512], F32, tag="ps")
                for mt in range(nmt):
                    nc.tensor.matmul(
                        ps,
                        w_sb[:, mt, :],
                        xt_sb[mt][:, cs],
                        start=(mt == 0),
                        stop=(mt == nmt - 1),
                    )
                nc.vector.tensor_copy(t_sb[0:HD, cs], ps)
            for st in range(4 * ch, 4 * ch + 4):
                vps = scores_ps.tile([128, HD], F32, tag="ps")
                for mt in range(nmt):
                    nc.tensor.matmul(
                        vps,
                        xt_sb[mt][:, st * 128 : (st + 1) * 128],
                        wv_sb[:, mt, :],
                        start=(mt == 0),
                        stop=(mt == nmt - 1),
                    )
                nc.vector.tensor_copy(von[:, st, 0:HD], vps)

        # ---- main loop over query chunks ---------------------------------
        pending = []  # deferred per-s-tile output-projection closures

        def emit_epilogue(q0):
            """outT evac + denominator prep for the chunk ending now; the
            8 output projections are deferred into the next chunk."""
            outw = outw_pool.tile([128, qcw], F32R, tag="outw", name="outw")
            nc.vector.tensor_copy(outw, psum_o[0])
            den_dram = dram_pool.tile([1, qcw], F32, tag="den_dram", name="dd")
            nc.sync.dma_start(out=den_dram, in_=outw[HD : HD + 1, :].bitcast(F32))
            den_col = small_pool.tile([128, qcw // 128], F32, tag="den_col", name="dc")
            nc.sync.dma_start(
                out=den_col, in_=den_dram.rearrange("p (c q) -> (p q) c", q=128)
            )
            recip = small_pool.tile([128, qcw // 128], F32, tag="recip", name="rc")
            nc.vector.reciprocal(recip, den_col)

            def mk(t):
                def emit():
                    yp = y_ps.tile([128, D], F32, tag="yp", name="yp")
                    nc.tensor.matmul(
                        yp,
                        outw[:, t * 128 : (t + 1) * 128],
                        wo_sb,
                        start=True,
                        stop=True,
                    )
                    y_sb = y_sb_pool.tile([128, D], F32, tag="y_sb", name="ysb")
                    nc.vector.tensor_copy(y_sb, yp)
                    nc.vector.tensor_scalar_mul(y_sb, y_sb, recip[:, t : t + 1])
                    row0 = q0 + t * 128
                    nc.sync.dma_start(out=y_d[row0 : row0 + 128, :], in_=y_sb)

                return emit

            return [mk(t) for t in range(qcw // 128)]

        psum_o = [None]
        for qc in range(nqc):
            q0 = qc * qcw
            psum_o[0] = out_ps.tile([128, qcw], F32, tag="po", name="po")

            def pv(kt, attn_t):
                for j in range(n_qmm):
                    js = slice(j * qmm_w, (j + 1) * qmm_w)
                    nc.tensor.matmul(
                        psum_o[0][:, js],
                        von[:, kt, :],
                        attn_t[:, js],
                        start=(kt == 0),
                        stop=(kt == nkt - 1),
                    )

            prev = None
            for kt in range(nkt):
                psum_s = scores_ps.tile([128, qcw], F32, tag="ps", name="ps")
                for j in range(n_qmm):
                    js = slice(j * qmm_w, (j + 1) * qmm_w)
                    nc.tensor.matmul(
                        psum_s[:, js],
                        kt0[:, kt * 128 : (kt + 1) * 128],
                        qt0[:, q0 + j * qmm_w : q0 + (j + 1) * qmm_w],
                        start=True,
                        stop=True,
                    )
                attn_t = attn_pool.tile([128, qcw], F32R, tag="attn", name="at")
                nc.scalar.activation(out=attn_t, in_=psum_s, func=EXP, scale=SCALE)
                if prev is not None:
                    pv(*prev)
                prev = (kt, attn_t)
                if kt < len(pending):
                    pending[kt]()
            pv(*prev)
            pending = emit_epilogue(q0)
        for fn in pending:
            fn()


def build_program(s=S):
    nc = bacc.Bacc(
        "TRN2", target_bir_lowering=False, debug=False, num_devices=N_CORES
    )
    ins = {
        "xt": nc.dram_tensor("xt", [D, s], F32R, kind="ExternalInput").ap(),
        "wq": nc.dram_tensor("wq", [D, HD], F32R, kind="ExternalInput").ap(),
        "wk": nc.dram_tensor("wk", [D, HD], F32R, kind="ExternalInput").ap(),
        "wv": nc.dram_tensor("wv", [D, HD], F32R, kind="ExternalInput").ap(),
        "wo": nc.dram_tensor("wo", [HD, D], F32R, kind="ExternalInput").ap(),
        "zeros64": nc.dram_tensor("zeros64", [64, s], F32R, kind="ExternalInput").ap(),
        "vinit": nc.dram_tensor("vinit", [128, s // 128, 64], F32R, kind="ExternalInput").ap(),
    }
    outs = {"y": nc.dram_tensor("y", [s, D], F32, kind="ExternalOutput").ap()}
    with tile.TileContext(nc) as tc:
        attention_kernel(tc, outs, ins, s=s)
    nc.compile()
    return nc


_NC_CACHE = {}


_Z64 = np.zeros((64, S), dtype=np.float32)
_VINIT = np.zeros((128, S // 128, 64), dtype=np.float32)
_VINIT[:, :, 0] = 1.0


def make_in_maps(x, Wq, Wk, Wv, Wo):
    """Host-side sharding: slice per-head weights, transpose layouts."""
    xt = np.ascontiguousarray(x.reshape(-1, D).T).astype(np.float32)
    in_maps = []
    for h in range(N_CORES):
        r = slice(h * HD, (h + 1) * HD)
        in_maps.append(
            {
                "xt": xt,
                "wq": np.ascontiguousarray(Wq[r, :].T).astype(np.float32),
                "wk": np.ascontiguousarray(Wk[r, :].T).astype(np.float32),
                "wv": np.ascontiguousarray(Wv[r, :].T).astype(np.float32),
                "wo": np.ascontiguousarray(Wo[:, r].T).astype(np.float32),
                "zeros64": _Z64,
                "vinit": _VINIT,
            }
        )
    return in_maps


def run(x, Wq, Wk, Wv, Wo, trace=False):
    if S not in _NC_CACHE:
        _NC_CACHE[S] = build_program(S)
    nc = _NC_CACHE[S]
    in_maps = make_in_maps(x, Wq, Wk, Wv, Wo)
    res = run_bass_kernel_spmd(
        nc, in_maps, core_ids=list(range(N_CORES)), trace=trace
    )
    y = res.results[0]["y"].astype(np.float64)
    for c in range(1, N_CORES):
        y += res.results[c]["y"]
    return y.astype(np.float32).reshape(1, S, D), res


def kernel(x, Wq, Wk, Wv, Wo):
    y, _ = run(
        np.asarray(x, dtype=np.float32),
        np.asarray(Wq, dtype=np.float32),
        np.asarray(Wk, dtype=np.float32),
        np.asarray(Wv, dtype=np.float32),
        np.asarray(Wo, dtype=np.float32),
    )
    return y
